# revision 56
# baseline (speedup 1.0000x reference)
"""Block-sparse attention (SageAttention-style mean-similarity top-k) on 8 TRN2 NeuronCores.

Sharding: 16 heads tensor-parallel across 8 cores (2 heads/core).
  - qkv weight column-sharded per core (its 2 heads' q/k/v rows, pre-transposed on host)
  - block selection + block-sparse attention fully local per head
  - proj weight row-sharded: each core computes the full-shape PARTIAL product
    o_local @ projW[:, c_slice].T (+ bias on core 0 only); the host unshard step
    sums the 8 partials (the row-parallel reduction).

v6 = the v1 structure (pools / emit order / sync-queue DMAs are load-bearing:
every pipeline restructure measured slower) with only two proven changes:
  - fp16 end-to-end: x uploaded fp16 (CPU-sim verified the f32-sum-of-fp16
    selection keeps all top-k picks identical; bf16 would flip blocks and fail),
    weights host-cast to fp16, e/o/obounce fp16. Removes the 27us ACT x-cast
    and halves x DMA bytes. HW-verified numerics (rel err 4e-4 in v4).
  - host pre-chunks x into SBUF-partition-major [nch, p, kc, m] so each chunk
    loads with 128 8KB descriptors instead of 1024 1KB ones (the x DMA was
    descriptor-generation-bound, ~6us per 1MB).

v7-v12 (this version, 273360ns vs v6's 314469ns; HW-traced at each step):
  - proj: own PSUM pool, bias on host, fp16 partials, CHQ=4, m-tiles age-gated
    and interleaved 2/iteration into the NEXT chunk (in-order PE queue never
    waits on the DVE PSUM->SBUF copies); final chunks drain through a 4-deep
    PSUM ring (opened after the loop pools close) with copies alternating
    ACT/DVE.
  - scores: K=64 lhsT gets no FWL (105ns/LDW measured) -> one K=128 MM per
    selected block covers BOTH heads via a block-diagonal zero-padded q rhs
    [128,256] (qzzT); out cols 0:128=h0, 128:256=h1. 16 MMs/qb at the N=256
    streaming rate.
  - o^T built on-chip with PE transposes of onorm (no HBM obounce roundtrip).
  - exp: 8 tiles/qb; e1 of the EVEN quarters runs on DVE as Schraudolph
    fast-exp (bitcast_fp16(int16(s*A+B))) pair-parallel with ACT's e0 so the
    2-buffer scores-PSUM ring gate halves; sawtooth cancels in the softmax
    ratio (+4e-3 rel err, gate is 2e-2). Odd quarters on DVE measured SLOWER
    (head-blocks reciprocal/normalize in the strict-FIFO DVE queue).
  - software pipeline: kg gather leads 1 iteration (heads the gpsimd queue),
    o-phase trails 3, o^T transpose trails 5 (DVE tail slack).
  - x block sums: one tensor_reduce per chunk (was 8) - saves ~26us DVE.

Per-core device pipeline: x^T fp16 -> block sums (DVE, f32) -> qm/km/sim (f32
PE) -> top-16 via max8/max_index; qkv matmuls (fp16 PE); per query block:
ap_gather pulls the 16 selected k/v blocks; two-head scores per block via the
qzzT trick; exp on ACT+DVE straight from PSUM; o = (e^T)^T v_sel with a
gathered ones column as softmax denominator; per-partition normalize; PE
transpose -> chunk ot tiles -> projection partials streamed out fp16.
"""

import os
import sys

for _p in ("/opt/trn_rl_repo", "/root/.axon_site/_ro/trn_rl_repo"):
    if os.path.isdir(_p) and _p not in sys.path:
        sys.path.insert(0, _p)

import numpy as np

import concourse.bass as bass
import concourse.bacc as bacc
import concourse.tile as tile
import concourse.mybir as mybir
from concourse.bass_utils import run_bass_kernel_spmd
from concourse.library_config import ap_gather as ap_gather_lib

# problem constants
N = 4096          # sequence length
C = 1024          # model dim
H = 16            # heads
D = 64            # head dim
BLK = 128         # block size
NB = N // BLK     # 32 blocks
TOPK = 16         # int(0.5 * NB)
NCORES = 8
HPC = H // NCORES  # 2 heads per core
SCALE = D ** -0.5  # 0.125
# Schraudolph fast-exp constants for fp16: bitcast_fp16(int16(s*A + B)) ~=
# exp(s*SCALE); the +-3% sawtooth cancels in the softmax ratio (verified
# offline: ~3e-3 added rel err for 2 of 8 tiles per query block)
SCHRAU_A = SCALE * 1024.0 / float(np.log(2.0))
SCHRAU_B = 15360.0 - 44.0

F32 = mybir.dt.float32
F16 = mybir.dt.float16
I16 = mybir.dt.int16
U32 = mybir.dt.uint32

_CACHE = {}


def _build():
    nc = bacc.Bacc("TRN2", target_bir_lowering=False, debug=False,
                   num_devices=NCORES)

    KC = C // 128  # 8 contraction tiles

    xT5 = nc.dram_tensor("xT5", [8, 128, KC, 512], F16, kind="ExternalInput")
    wqkvT = nc.dram_tensor("wqkvT", [128, KC, 384], F16, kind="ExternalInput")
    projWT = nc.dram_tensor("projWT", [2 * D, C], F16, kind="ExternalInput")
    ident64 = nc.dram_tensor("ident64", [64, 64], F32, kind="ExternalInput")
    ident128 = nc.dram_tensor("ident128", [128, 128], F16, kind="ExternalInput")
    erep = nc.dram_tensor("erep", [16, 128], F32, kind="ExternalInput")
    out_ext = nc.dram_tensor("out", [C, N], F16, kind="ExternalOutput")

    with tile.TileContext(nc) as tc:
        nc.gpsimd.load_library(ap_gather_lib)

        with tc.tile_pool(name="persist", bufs=1) as pp:
            # ---- weights ----
            wqkv_h = pp.tile([128, KC, 384], F16)
            nc.sync.dma_start(wqkv_h[:], wqkvT.ap())
            projW_h = pp.tile([128, C], F16)            # [c_local, j]
            nc.gpsimd.dma_start(projW_h[:], projWT.ap())
            id64 = pp.tile([64, 64], F32)
            nc.gpsimd.dma_start(id64[:], ident64.ap())
            id128 = pp.tile([128, 128], F16)
            nc.gpsimd.dma_start(id128[:], ident128.ap())
            erep_sb = pp.tile([16, 128], F32)
            nc.gpsimd.dma_start(erep_sb[:], erep.ap())

            # ---- selection block sums, built per chunk straight from the
            # q/k PSUM tiles (sum over tokens of PE-computed q/k == W @ x
            # block-sum; fp16-W noise flips 1 of 512 top-k sets, +5e-3 rel
            # err verified offline) ----
            qm_sb = pp.tile([128, NB], F32)
            km_sb = pp.tile([128, NB], F32)

            # ---- QKV (fp16) ----
            # qzzT[p, qb, c]: block-diagonal q for K=128 two-head score MMs:
            #   rows 0:64  hold h0's q dims in cols 0:128   (zeros in 128:256)
            #   rows 64:128 hold h1's q dims in cols 128:256 (zeros in 0:128)
            # so lhsT=kg[:, j, :] (both heads stacked on K) gives
            # out[key, 0:128]=h0 scores, out[key, 128:256]=h1 scores.
            qzzT = pp.tile([128, NB, 256], F16)
            kT = pp.tile([128, NB, BLK], F16)   # contiguous == [128, N]
            v0 = pp.tile([128, NB, 66], F16)
            v1 = pp.tile([128, NB, 66], F16)

            def _emit_memsets():
                # emitted AFTER the per-chunk reduces so the DVE-serial
                # reduce chain (which gates selection) starts immediately;
                # all consumers (scores qzzT zeros, gather ones-cols) are
                # in phase B, long after these run in the DVE queue
                nc.vector.memset(qzzT[0:64, :, 128:256], 0.0)
                nc.vector.memset(qzzT[64:128, :, 0:128], 0.0)
                nc.vector.memset(v0[:, :, 64:66], 0.0)
                nc.vector.memset(v1[:, :, 64:66], 0.0)
                nc.vector.memset(v0[:, :, 64:65], 1.0)
                nc.vector.memset(v1[:, :, 64:65], 1.0)

            with tc.tile_pool(name="xload", bufs=3) as xp, \
                 tc.tile_pool(name="qkps", bufs=3, space="PSUM") as qp, \
                 tc.tile_pool(name="vps", bufs=3, space="PSUM") as vp:
                for nch in range(8):
                    lo, hi = nch * 512, (nch + 1) * 512
                    xf = xp.tile([128, KC, 512], F16, tag="xf", name=f"xf_{nch}")
                    nc.sync.dma_start(xf[:], xT5.ap()[nch])
                    for mt in (0, 1):
                        ps = qp.tile([128, 512], F32, tag="qk")
                        for kc in range(KC):
                            nc.tensor.matmul(
                                ps[:], lhsT=wqkv_h[:, kc, mt * 128:(mt + 1) * 128],
                                rhs=xf[:, kc, :],
                                start=(kc == 0), stop=(kc == KC - 1))
                        nc.vector.tensor_reduce(
                            (qm_sb if mt == 0 else km_sb)[:, nch * 4:(nch + 1) * 4],
                            ps[:].rearrange("p (b t) -> p b t", t=BLK),
                            axis=mybir.AxisListType.X, op=mybir.AluOpType.add)
                        if mt == 0:
                            nc.scalar.copy(
                                qzzT[0:64, nch * 4:(nch + 1) * 4, 0:128],
                                ps[0:64, :].rearrange("p (a b) -> p a b", b=128))
                            nc.scalar.copy(
                                qzzT[64:128, nch * 4:(nch + 1) * 4, 128:256],
                                ps[64:128, :].rearrange("p (a b) -> p a b", b=128))
                        else:
                            nc.scalar.copy(
                                kT[:].rearrange("p a b -> p (a b)")[:, lo:hi],
                                ps[:])
                    if nch == 7:
                        _emit_memsets()
                    for j4 in range(4):
                        nt = 4 * nch + j4
                        psv = vp.tile([128, 128], F32, tag="v")
                        for kc in range(KC):
                            nc.tensor.matmul(psv[:], lhsT=xf[:, kc, j4 * 128:(j4 + 1) * 128],
                                             rhs=wqkv_h[:, kc, 256:384],
                                             start=(kc == 0), stop=(kc == KC - 1))
                        nc.scalar.copy(v0[:, nt, 0:64], psv[:, 0:64])
                        nc.scalar.copy(v1[:, nt, 0:64], psv[:, 64:128])

            # ---- block-mean similarity + top-k selection (f32) ----
            kidx = pp.tile([128, NB], I16)
            vidx0 = pp.tile([128, NB], I16)
            vidx1 = pp.tile([128, NB], I16)
            with tc.tile_pool(name="selps", bufs=2, space="PSUM") as sp, \
                 tc.tile_pool(name="selsb", bufs=2) as sb:
                sim_ps = sp.tile([64, NB], F32, tag="sim")
                for h in range(HPC):
                    nc.tensor.matmul(sim_ps[h * 32:(h + 1) * 32, :],
                                     lhsT=qm_sb[h * 64:(h + 1) * 64, :],
                                     rhs=km_sb[h * 64:(h + 1) * 64, :],
                                     start=True, stop=True)
                sim2 = sb.tile([64, NB], F32, tag="sim2")
                nc.vector.tensor_copy(sim2[:], sim_ps[:])

                vals0 = sb.tile([64, 8], F32, tag="v0")
                idx0 = sb.tile([64, 8], U32, tag="i0")
                pun = sb.tile([64, NB], F32, tag="pun")
                vals1 = sb.tile([64, 8], F32, tag="v1")
                idx1 = sb.tile([64, 8], U32, tag="i1")
                nc.vector.max(vals0[:], sim2[:])
                nc.vector.max_index(idx0[:], vals0[:], sim2[:])
                nc.vector.match_replace(out=pun[:], in_to_replace=vals0[:],
                                        in_values=sim2[:], imm_value=-1e30)
                nc.vector.max(vals1[:], pun[:])
                nc.vector.max_index(idx1[:], vals1[:], pun[:])

                idxf = sb.tile([64, TOPK], F32, tag="idxf")
                nc.vector.tensor_copy(idxf[:, 0:8], idx0[:])
                nc.vector.tensor_copy(idxf[:, 8:16], idx1[:])

                selT_ps = sp.tile([TOPK, 64], F32, tag="selT")
                nc.tensor.transpose(selT_ps[:], idxf[:], id64[:])
                selT = sb.tile([TOPK, 64], F32, tag="selTsb")
                nc.vector.tensor_copy(selT[:], selT_ps[:])

                # replicate selT rows to all 16-partition groups via one matmul:
                # rep[m, n] = selT[m % 16, n]
                rep_ps = sp.tile([128, 64], F32, tag="rep")
                nc.tensor.matmul(rep_ps[:], lhsT=erep_sb[:], rhs=selT[:],
                                 start=True, stop=True)
                nc.vector.tensor_copy(kidx[0:64, :], rep_ps[0:64, 0:32])
                nc.vector.tensor_copy(kidx[64:128, :], rep_ps[64:128, 32:64])
                nc.vector.tensor_copy(vidx0[:], rep_ps[:, 0:32])
                nc.vector.tensor_copy(vidx1[:], rep_ps[:, 32:64])

            # ---- main loop: sparse attention + chunked projection partials ----
            CHQ = 4                    # query blocks per projection chunk
            CHT = CHQ * BLK            # 512 tokens per chunk
            with tc.tile_pool(name="gather", bufs=12) as gp, \
                 tc.tile_pool(name="escore", bufs=32) as ep, \
                 tc.tile_pool(name="otp", bufs=2) as otp, \
                 tc.tile_pool(name="prout", bufs=4) as pr, \
                 tc.tile_pool(name="osb", bufs=8) as ob:

                # o^T is built on-chip: a PE transpose per query block (trails
                # the o-phase by one iteration) accumulates each chunk's
                # [c_local, token] tile; proj m-tiles are then emitted 2 per
                # iteration across the NEXT chunk so the in-order PE queue
                # never waits on the DVE PSUM->SBUF copies.
                pending_proj = []
                ots = {}
                onorms = {}

                def _emit_proj_m(c, ot, m, pool, copy_eng=None):
                    pj = pool.tile([128, CHT], F32, tag="pj", name=f"pj_{c}_{m}")
                    nc.tensor.matmul(pj[:], lhsT=projW_h[:, m * 128:(m + 1) * 128],
                                     rhs=ot[:], start=True, stop=True)
                    po = pr.tile([128, CHT], F16, tag="po", name=f"po_{c}_{m}")
                    if copy_eng is None:
                        nc.vector.tensor_copy(po[:], pj[:])
                    else:
                        copy_eng(po[:], pj[:])
                    nc.sync.dma_start(
                        out_ext.ap()[m * 128:(m + 1) * 128, c * CHT:(c + 1) * CHT],
                        po[:])

                def emit_tr(qb):
                    if qb < 0 or qb not in onorms:
                        return
                    c = qb // CHQ
                    if qb % CHQ == 0:
                        ots[c] = otp.tile([128, CHT], F16, tag="ot",
                                          name=f"ot_{c}")
                    onorm = onorms.pop(qb)
                    otr = trp.tile([128, BLK], F16, tag="otr", name=f"otr_{qb}")
                    nc.tensor.transpose(otr[:], onorm[:], id128[:])
                    nc.vector.tensor_copy(
                        ots[c][:, (qb % CHQ) * BLK:(qb % CHQ + 1) * BLK],
                        otr[:])
                    if qb % CHQ == CHQ - 1:
                        ot = ots.pop(c)
                        for m in range(KC):
                            pending_proj.append((c, ot, m))

                def _pop_proj(n, qb):
                    # age gate: chunk c finishes its last transpose at
                    # iteration (c+1)*CHQ+2; only pop from (c+1)*CHQ+3 on
                    for _ in range(min(n, len(pending_proj))):
                        c, ot, m = pending_proj[0]
                        if qb < (c + 1) * CHQ + 5:
                            return
                        pending_proj.pop(0)
                        _emit_proj_m(c, ot, m, pps)

                state = {}

                kgs = {}

                def emit_kg(qb):
                    # kg is prefetched one iteration ahead of its scores and
                    # emitted FIRST so it leads the in-order gpsimd queue
                    kg = gp.tile([128, TOPK, BLK], F16, tag="kg",
                                 name=f"kg_{qb}")
                    nc.gpsimd.ap_gather(kg[:], kT[:], kidx[:, qb:qb + 1],
                                        channels=128, num_elems=NB, d=BLK, num_idxs=TOPK)
                    kgs[qb] = kg

                def emit_vg(qb):
                    vg0 = gp.tile([128, TOPK, 66], F16, tag="vg0",
                                  name=f"vg0_{qb}")
                    nc.gpsimd.ap_gather(vg0[:], v0[:], vidx0[:, qb:qb + 1],
                                        channels=128, num_elems=NB, d=66, num_idxs=TOPK)
                    vg1 = gp.tile([128, TOPK, 66], F16, tag="vg1",
                                  name=f"vg1_{qb}")
                    nc.gpsimd.ap_gather(vg1[:], v1[:], vidx1[:, qb:qb + 1],
                                        channels=128, num_elems=NB, d=66, num_idxs=TOPK)
                    onorm = ob.tile([128, 2 * D], F16, tag="onorm",
                                    name=f"on_{qb}")
                    state[qb] = ([[None] * 4, [None] * 4], kgs.pop(qb), vg0, vg1,
                                 onorm)

                def emit_scores_quarter(qb, quarter):
                    # one K=128 MM per selected block covers BOTH heads via the
                    # block-diagonal qzzT rhs: out cols 0:128=h0, 128:256=h1.
                    # (K=64 lhsT weights don't get FWL; this keeps the PE at
                    # the N=256 streaming rate instead of the 107ns LDW rate.)
                    etiles, kg, vg0, vg1, onorm = state[qb]
                    s = spp.tile([128, 4, 256], F32, tag="s",
                                 name=f"s_{qb}_{quarter}")
                    for jj in range(4):
                        j = quarter * 4 + jj
                        nc.tensor.matmul(s[:, jj, :], lhsT=kg[:, j, :],
                                         rhs=qzzT[:, qb, :],
                                         start=True, stop=True)
                    e0 = ep.tile([128, 4, 128], F16, tag="e",
                                 name=f"e0_{qb}_{quarter}")
                    nc.scalar.activation(e0[:], s[:, :, 0:128],
                                         mybir.ActivationFunctionType.Exp, scale=SCALE)
                    e1 = ep.tile([128, 4, 128], F16, tag="e",
                                 name=f"e1_{qb}_{quarter}")
                    if quarter in (0, 2):
                        # the scores PSUM ring (2 bufs) makes quarter i+2 wait
                        # on BOTH exps of quarter i; running e1 of the even
                        # quarters on DVE (Schraudolph) pair-parallel with
                        # ACT's e0 halves that gate latency; at these emission
                        # points the DVE queue is idle so nothing head-blocks
                        # (offloading the odd quarters too measured SLOWER:
                        # their schrau head-blocks reciprocal/normalize)
                        nc.vector.tensor_scalar(
                            e1[:].bitcast(I16), s[:, :, 128:256],
                            SCHRAU_A, SCHRAU_B,
                            op0=mybir.AluOpType.mult, op1=mybir.AluOpType.add)
                    else:
                        nc.scalar.activation(e1[:], s[:, :, 128:256],
                                             mybir.ActivationFunctionType.Exp,
                                             scale=SCALE)
                    etiles[1][quarter] = e1
                    etiles[0][quarter] = e0

                def emit_o(qb, heads=(0, 1)):
                    if qb not in state:
                        return
                    etiles, kg, vg0, vg1, onorm = state[qb]
                    for h in heads:
                        vg = vg0 if h == 0 else vg1
                        o_ps = opp.tile([128, D + 1], F32, tag="o",
                                        name=f"o_{qb}_{h}")
                        for j in range(TOPK):
                            nc.tensor.matmul(o_ps[:],
                                             lhsT=etiles[h][j // 4][:, j % 4, :],
                                             rhs=vg[:, j, 0:D + 1],
                                             start=(j == 0), stop=(j == TOPK - 1))
                        rec = ob.tile([128, 1], F32, tag="rec", name=f"r_{qb}_{h}")
                        nc.vector.reciprocal(rec[:], o_ps[:, D:D + 1])
                        nc.vector.tensor_scalar(onorm[:, h * D:(h + 1) * D],
                                                o_ps[:, 0:D], rec[:], None,
                                                op0=mybir.AluOpType.mult)
                    if heads[-1] == 1:
                        state.pop(qb)
                        onorms[qb] = onorm

                # software pipeline: o-phase trails scores by one iteration and
                # is interleaved BETWEEN the two score halves so attn-v matmuls
                # cover the exp latency (sps bufs=2 reuses the half-0 PSUM
                # banks for half 1 only after its exp drains them)
                # software pipeline: kg leads by 1 iteration, the o-phase
                # trails by 3 and the o^T PE transpose by 5, giving the DVE
                # tail (schrau e1 / normalize) a full extra iteration of
                # slack before the PE consumes its outputs
                with tc.tile_pool(name="sps", bufs=2, space="PSUM") as spp, \
                     tc.tile_pool(name="ops", bufs=2, space="PSUM") as opp, \
                     tc.tile_pool(name="pps", bufs=1, space="PSUM") as pps, \
                     tc.tile_pool(name="trp", bufs=1, space="PSUM") as trp:
                    emit_kg(0)
                    for qb in range(NB):
                        if qb + 1 < NB:
                            emit_kg(qb + 1)
                        emit_vg(qb)
                        emit_scores_quarter(qb, 0)
                        emit_scores_quarter(qb, 1)
                        emit_o(qb - 3, heads=(0,))
                        emit_tr(qb - 5)
                        _pop_proj(2, qb)
                        emit_scores_quarter(qb, 2)
                        emit_scores_quarter(qb, 3)
                        emit_o(qb - 3, heads=(1,))
                        _pop_proj(1, qb)
                    for qb in (NB - 3, NB - 2, NB - 1):
                        emit_o(qb, heads=(0,))
                        emit_o(qb, heads=(1,))
                    for qb in range(NB - 5, NB):
                        emit_tr(qb)

                # the in-loop PSUM pools are closed now; drain the remaining
                # proj m-tiles through a 4-deep ring with the PSUM->SBUF
                # copies alternating DVE/ACT so the tail is copy-throughput
                # bound on two engines instead of a serial 1-buffer chain
                with tc.tile_pool(name="dps", bufs=4, space="PSUM") as dps:
                    for di in range(len(pending_proj)):
                        c, ot, m = pending_proj.pop(0)
                        _emit_proj_m(c, ot, m, dps,
                                     copy_eng=(nc.scalar.copy if di % 2
                                               else nc.vector.tensor_copy))

    nc.compile()
    return nc


def _prep_inputs(x, qkv_w, proj_w, proj_b):
    x = np.asarray(x, dtype=np.float32)
    qkv_w = np.asarray(qkv_w, dtype=np.float32)
    proj_w = np.asarray(proj_w, dtype=np.float32)
    proj_b = np.asarray(proj_b, dtype=np.float32)

    xT = x[0].T.astype(np.float16)                         # [C, N]
    # xT.reshape(KC_a, 128_p, 8_nch, 512_m) -> [nch, p, a, m] partition-major
    xT5 = np.ascontiguousarray(
        xT.reshape(8, 128, 8, 512).transpose(2, 1, 0, 3))
    ident64 = np.eye(64, dtype=np.float32)
    ident128 = np.eye(128, dtype=np.float16)
    erep = (np.arange(128)[None, :] % 16 == np.arange(16)[:, None]).astype(np.float32)
    in_maps = []
    for i in range(NCORES):
        h0 = HPC * i
        rows = []
        for part in range(3):                              # q, k, v row groups
            base = part * C + h0 * D
            rows.append(qkv_w[base:base + HPC * D, :])
        wqkv = np.concatenate(rows, axis=0)                # [384, C]
        wqkvT_np = np.ascontiguousarray(wqkv.T)            # [C, 384]
        wq5 = np.ascontiguousarray(
            wqkvT_np.reshape(8, 128, 384).transpose(1, 0, 2))
        cslice = slice(i * 2 * D, (i + 1) * 2 * D)
        in_maps.append({
            "xT5": xT5,
            "wqkvT": wq5.astype(np.float16),
            # [c_local, j]: rows = this core's 128 c-dims, cols = all 1024 j
            "projWT": np.ascontiguousarray(proj_w[:, cslice].T).astype(np.float16),
            "ident64": ident64,
            "ident128": ident128,
            "erep": erep,
        })
    return in_maps, proj_b


def kernel(x, qkv_w, proj_w, proj_b, _trace=False):
    if "nc" not in _CACHE:
        _CACHE["nc"] = _build()
    nc = _CACHE["nc"]
    in_maps, bias = _prep_inputs(x, qkv_w, proj_w, proj_b)
    res = run_bass_kernel_spmd(nc, in_maps, core_ids=list(range(NCORES)),
                               trace=_trace)
    outT = res.results[0]["out"].astype(np.float32)
    for i in range(1, NCORES):
        outT += res.results[i]["out"].astype(np.float32)
    out = np.ascontiguousarray(outT.T).reshape(1, N, C).astype(np.float32)
    out += bias[None, None, :]
    if _trace:
        _CACHE["last_exec_time_ns"] = res.exec_time_ns
        _CACHE["last_results"] = res
    return out



# revision 58
# speedup vs baseline: 1.0030x; 1.0030x over previous
"""Block-sparse attention (SageAttention-style mean-similarity top-k) on 8 TRN2 NeuronCores.

Sharding: 16 heads tensor-parallel across 8 cores (2 heads/core).
  - qkv weight column-sharded per core (its 2 heads' q/k/v rows, pre-transposed on host)
  - block selection + block-sparse attention fully local per head
  - proj weight row-sharded: each core computes the full-shape PARTIAL product
    o_local @ projW[:, c_slice].T (+ bias on core 0 only); the host unshard step
    sums the 8 partials (the row-parallel reduction).

v6 = the v1 structure (pools / emit order / sync-queue DMAs are load-bearing:
every pipeline restructure measured slower) with only two proven changes:
  - fp16 end-to-end: x uploaded fp16 (CPU-sim verified the f32-sum-of-fp16
    selection keeps all top-k picks identical; bf16 would flip blocks and fail),
    weights host-cast to fp16, e/o/obounce fp16. Removes the 27us ACT x-cast
    and halves x DMA bytes. HW-verified numerics (rel err 4e-4 in v4).
  - host pre-chunks x into SBUF-partition-major [nch, p, kc, m] so each chunk
    loads with 128 8KB descriptors instead of 1024 1KB ones (the x DMA was
    descriptor-generation-bound, ~6us per 1MB).

v7-v12 (this version, 273360ns vs v6's 314469ns; HW-traced at each step):
  - proj: own PSUM pool, bias on host, fp16 partials, CHQ=4, m-tiles age-gated
    and interleaved 2/iteration into the NEXT chunk (in-order PE queue never
    waits on the DVE PSUM->SBUF copies); final chunks drain through a 4-deep
    PSUM ring (opened after the loop pools close) with copies alternating
    ACT/DVE.
  - scores: K=64 lhsT gets no FWL (105ns/LDW measured) -> one K=128 MM per
    selected block covers BOTH heads via a block-diagonal zero-padded q rhs
    [128,256] (qzzT); out cols 0:128=h0, 128:256=h1. 16 MMs/qb at the N=256
    streaming rate.
  - o^T built on-chip with PE transposes of onorm (no HBM obounce roundtrip).
  - exp: 8 tiles/qb; e1 of the EVEN quarters runs on DVE as Schraudolph
    fast-exp (bitcast_fp16(int16(s*A+B))) pair-parallel with ACT's e0 so the
    2-buffer scores-PSUM ring gate halves; sawtooth cancels in the softmax
    ratio (+4e-3 rel err, gate is 2e-2). Odd quarters on DVE measured SLOWER
    (head-blocks reciprocal/normalize in the strict-FIFO DVE queue).
  - software pipeline: kg gather leads 1 iteration (heads the gpsimd queue),
    o-phase trails 3, o^T transpose trails 5 (DVE tail slack).
  - x block sums: one tensor_reduce per chunk (was 8) - saves ~26us DVE.

Per-core device pipeline: x^T fp16 -> block sums (DVE, f32) -> qm/km/sim (f32
PE) -> top-16 via max8/max_index; qkv matmuls (fp16 PE); per query block:
ap_gather pulls the 16 selected k/v blocks; two-head scores per block via the
qzzT trick; exp on ACT+DVE straight from PSUM; o = (e^T)^T v_sel with a
gathered ones column as softmax denominator; per-partition normalize; PE
transpose -> chunk ot tiles -> projection partials streamed out fp16.
"""

import os
import sys

for _p in ("/opt/trn_rl_repo", "/root/.axon_site/_ro/trn_rl_repo"):
    if os.path.isdir(_p) and _p not in sys.path:
        sys.path.insert(0, _p)

import numpy as np

import concourse.bass as bass
import concourse.bacc as bacc
import concourse.tile as tile
import concourse.mybir as mybir
from concourse.bass_utils import run_bass_kernel_spmd
from concourse.library_config import ap_gather as ap_gather_lib

# problem constants
N = 4096          # sequence length
C = 1024          # model dim
H = 16            # heads
D = 64            # head dim
BLK = 128         # block size
NB = N // BLK     # 32 blocks
TOPK = 16         # int(0.5 * NB)
NCORES = 8
HPC = H // NCORES  # 2 heads per core
SCALE = D ** -0.5  # 0.125
# Schraudolph fast-exp constants for fp16: bitcast_fp16(int16(s*A + B)) ~=
# exp(s*SCALE); the +-3% sawtooth cancels in the softmax ratio (verified
# offline: ~3e-3 added rel err for 2 of 8 tiles per query block)
SCHRAU_A = SCALE * 1024.0 / float(np.log(2.0))
SCHRAU_B = 15360.0 - 44.0

F32 = mybir.dt.float32
F16 = mybir.dt.float16
I16 = mybir.dt.int16
U32 = mybir.dt.uint32

_CACHE = {}


def _build():
    nc = bacc.Bacc("TRN2", target_bir_lowering=False, debug=False,
                   num_devices=NCORES)

    KC = C // 128  # 8 contraction tiles

    xT5 = nc.dram_tensor("xT5", [8, 128, KC, 512], F16, kind="ExternalInput")
    wqkvT = nc.dram_tensor("wqkvT", [128, KC, 384], F16, kind="ExternalInput")
    projWT = nc.dram_tensor("projWT", [2 * D, C], F16, kind="ExternalInput")
    ident64 = nc.dram_tensor("ident64", [64, 64], F32, kind="ExternalInput")
    ident128 = nc.dram_tensor("ident128", [128, 128], F16, kind="ExternalInput")
    erep = nc.dram_tensor("erep", [16, 128], F32, kind="ExternalInput")
    out_ext = nc.dram_tensor("out", [C, N], F16, kind="ExternalOutput")

    with tile.TileContext(nc) as tc:
        nc.gpsimd.load_library(ap_gather_lib)

        with tc.tile_pool(name="persist", bufs=1) as pp:
            # ---- weights ----
            wqkv_h = pp.tile([128, KC, 384], F16)
            # first kc-half only; the second half queues behind x chunk 0 so
            # the first QKV matmuls start ~1us earlier
            nc.sync.dma_start(wqkv_h[:, 0:4, :], wqkvT.ap()[:, 0:4, :])
            projW_h = pp.tile([128, C], F16)            # [c_local, j]
            nc.gpsimd.dma_start(projW_h[:], projWT.ap())
            id64 = pp.tile([64, 64], F32)
            nc.gpsimd.dma_start(id64[:], ident64.ap())
            id128 = pp.tile([128, 128], F16)
            nc.gpsimd.dma_start(id128[:], ident128.ap())
            erep_sb = pp.tile([16, 128], F32)
            nc.gpsimd.dma_start(erep_sb[:], erep.ap())

            # ---- selection block sums, built per chunk straight from the
            # q/k PSUM tiles (sum over tokens of PE-computed q/k == W @ x
            # block-sum; fp16-W noise flips 1 of 512 top-k sets, +5e-3 rel
            # err verified offline) ----
            qm_sb = pp.tile([128, NB], F32)
            km_sb = pp.tile([128, NB], F32)

            # ---- QKV (fp16) ----
            # qzzT[p, qb, c]: block-diagonal q for K=128 two-head score MMs:
            #   rows 0:64  hold h0's q dims in cols 0:128   (zeros in 128:256)
            #   rows 64:128 hold h1's q dims in cols 128:256 (zeros in 0:128)
            # so lhsT=kg[:, j, :] (both heads stacked on K) gives
            # out[key, 0:128]=h0 scores, out[key, 128:256]=h1 scores.
            qzzT = pp.tile([128, NB, 256], F16)
            kT = pp.tile([128, NB, BLK], F16)   # contiguous == [128, N]
            v0 = pp.tile([128, NB, 66], F16)
            v1 = pp.tile([128, NB, 66], F16)

            def _emit_memsets():
                # emitted AFTER the per-chunk reduces so the DVE-serial
                # reduce chain (which gates selection) starts immediately;
                # all consumers (scores qzzT zeros, gather ones-cols) are
                # in phase B, long after these run in the DVE queue
                nc.vector.memset(qzzT[0:64, :, 128:256], 0.0)
                nc.vector.memset(qzzT[64:128, :, 0:128], 0.0)
                nc.vector.memset(v0[:, :, 64:66], 0.0)
                nc.vector.memset(v1[:, :, 64:66], 0.0)
                nc.vector.memset(v0[:, :, 64:65], 1.0)
                nc.vector.memset(v1[:, :, 64:65], 1.0)

            with tc.tile_pool(name="xload", bufs=3) as xp, \
                 tc.tile_pool(name="qkps", bufs=3, space="PSUM") as qp, \
                 tc.tile_pool(name="vps", bufs=3, space="PSUM") as vp:
                for nch in range(8):
                    lo, hi = nch * 512, (nch + 1) * 512
                    xf = xp.tile([128, KC, 512], F16, tag="xf", name=f"xf_{nch}")
                    nc.sync.dma_start(xf[:], xT5.ap()[nch])
                    if nch == 0:
                        nc.sync.dma_start(wqkv_h[:, 4:KC, :],
                                          wqkvT.ap()[:, 4:KC, :])
                    for mt in (0, 1):
                        ps = qp.tile([128, 512], F32, tag="qk")
                        for kc in range(KC):
                            nc.tensor.matmul(
                                ps[:], lhsT=wqkv_h[:, kc, mt * 128:(mt + 1) * 128],
                                rhs=xf[:, kc, :],
                                start=(kc == 0), stop=(kc == KC - 1))
                        nc.vector.tensor_reduce(
                            (qm_sb if mt == 0 else km_sb)[:, nch * 4:(nch + 1) * 4],
                            ps[:].rearrange("p (b t) -> p b t", t=BLK),
                            axis=mybir.AxisListType.X, op=mybir.AluOpType.add)
                        if mt == 0:
                            nc.scalar.copy(
                                qzzT[0:64, nch * 4:(nch + 1) * 4, 0:128],
                                ps[0:64, :].rearrange("p (a b) -> p a b", b=128))
                            nc.scalar.copy(
                                qzzT[64:128, nch * 4:(nch + 1) * 4, 128:256],
                                ps[64:128, :].rearrange("p (a b) -> p a b", b=128))
                        else:
                            nc.scalar.copy(
                                kT[:].rearrange("p a b -> p (a b)")[:, lo:hi],
                                ps[:])
                    if nch == 7:
                        _emit_memsets()
                    for j4 in range(4):
                        nt = 4 * nch + j4
                        psv = vp.tile([128, 128], F32, tag="v")
                        for kc in range(KC):
                            nc.tensor.matmul(psv[:], lhsT=xf[:, kc, j4 * 128:(j4 + 1) * 128],
                                             rhs=wqkv_h[:, kc, 256:384],
                                             start=(kc == 0), stop=(kc == KC - 1))
                        nc.scalar.copy(v0[:, nt, 0:64], psv[:, 0:64])
                        nc.scalar.copy(v1[:, nt, 0:64], psv[:, 64:128])

            # ---- block-mean similarity + top-k selection (f32) ----
            kidx = pp.tile([128, NB], I16)
            vidx0 = pp.tile([128, NB], I16)
            vidx1 = pp.tile([128, NB], I16)
            with tc.tile_pool(name="selps", bufs=2, space="PSUM") as sp, \
                 tc.tile_pool(name="selsb", bufs=2) as sb:
                sim_ps = sp.tile([64, NB], F32, tag="sim")
                for h in range(HPC):
                    nc.tensor.matmul(sim_ps[h * 32:(h + 1) * 32, :],
                                     lhsT=qm_sb[h * 64:(h + 1) * 64, :],
                                     rhs=km_sb[h * 64:(h + 1) * 64, :],
                                     start=True, stop=True)
                sim2 = sb.tile([64, NB], F32, tag="sim2")
                nc.vector.tensor_copy(sim2[:], sim_ps[:])

                vals0 = sb.tile([64, 8], F32, tag="v0")
                idx0 = sb.tile([64, 8], U32, tag="i0")
                pun = sb.tile([64, NB], F32, tag="pun")
                vals1 = sb.tile([64, 8], F32, tag="v1")
                idx1 = sb.tile([64, 8], U32, tag="i1")
                nc.vector.max(vals0[:], sim2[:])
                nc.vector.max_index(idx0[:], vals0[:], sim2[:])
                nc.vector.match_replace(out=pun[:], in_to_replace=vals0[:],
                                        in_values=sim2[:], imm_value=-1e30)
                nc.vector.max(vals1[:], pun[:])
                nc.vector.max_index(idx1[:], vals1[:], pun[:])

                idxf = sb.tile([64, TOPK], F32, tag="idxf")
                nc.vector.tensor_copy(idxf[:, 0:8], idx0[:])
                nc.vector.tensor_copy(idxf[:, 8:16], idx1[:])

                selT_ps = sp.tile([TOPK, 64], F32, tag="selT")
                nc.tensor.transpose(selT_ps[:], idxf[:], id64[:])
                selT = sb.tile([TOPK, 64], F32, tag="selTsb")
                nc.vector.tensor_copy(selT[:], selT_ps[:])

                # replicate selT rows to all 16-partition groups via one matmul:
                # rep[m, n] = selT[m % 16, n]
                rep_ps = sp.tile([128, 64], F32, tag="rep")
                nc.tensor.matmul(rep_ps[:], lhsT=erep_sb[:], rhs=selT[:],
                                 start=True, stop=True)
                nc.vector.tensor_copy(kidx[0:64, :], rep_ps[0:64, 0:32])
                nc.vector.tensor_copy(kidx[64:128, :], rep_ps[64:128, 32:64])
                nc.vector.tensor_copy(vidx0[:], rep_ps[:, 0:32])
                nc.vector.tensor_copy(vidx1[:], rep_ps[:, 32:64])

            # ---- main loop: sparse attention + chunked projection partials ----
            CHQ = 4                    # query blocks per projection chunk
            CHT = CHQ * BLK            # 512 tokens per chunk
            with tc.tile_pool(name="gather", bufs=9) as gp, \
                 tc.tile_pool(name="escore", bufs=32) as ep, \
                 tc.tile_pool(name="otp", bufs=2) as otp, \
                 tc.tile_pool(name="prout", bufs=4) as pr, \
                 tc.tile_pool(name="osb", bufs=8) as ob:

                # o^T is built on-chip: a PE transpose per query block (trails
                # the o-phase by one iteration) accumulates each chunk's
                # [c_local, token] tile; proj m-tiles are then emitted 2 per
                # iteration across the NEXT chunk so the in-order PE queue
                # never waits on the DVE PSUM->SBUF copies.
                pending_proj = []
                ots = {}
                onorms = {}

                def _emit_proj_m(c, ot, m, pool, copy_eng=None):
                    pj = pool.tile([128, CHT], F32, tag="pj", name=f"pj_{c}_{m}")
                    nc.tensor.matmul(pj[:], lhsT=projW_h[:, m * 128:(m + 1) * 128],
                                     rhs=ot[:], start=True, stop=True)
                    po = pr.tile([128, CHT], F16, tag="po", name=f"po_{c}_{m}")
                    if copy_eng is None:
                        nc.vector.tensor_copy(po[:], pj[:])
                    else:
                        copy_eng(po[:], pj[:])
                    nc.sync.dma_start(
                        out_ext.ap()[m * 128:(m + 1) * 128, c * CHT:(c + 1) * CHT],
                        po[:])

                def emit_tr(qb):
                    if qb < 0 or qb not in onorms:
                        return
                    c = qb // CHQ
                    if qb % CHQ == 0:
                        ots[c] = otp.tile([128, CHT], F16, tag="ot",
                                          name=f"ot_{c}")
                    onorm = onorms.pop(qb)
                    otr = trp.tile([128, BLK], F16, tag="otr", name=f"otr_{qb}")
                    nc.tensor.transpose(otr[:], onorm[:], id128[:])
                    nc.vector.tensor_copy(
                        ots[c][:, (qb % CHQ) * BLK:(qb % CHQ + 1) * BLK],
                        otr[:])
                    if qb % CHQ == CHQ - 1:
                        ot = ots.pop(c)
                        for m in range(KC):
                            pending_proj.append((c, ot, m))

                def _pop_proj(n, qb):
                    # age gate: chunk c finishes its last transpose at
                    # iteration (c+1)*CHQ+2; only pop from (c+1)*CHQ+3 on
                    for _ in range(min(n, len(pending_proj))):
                        c, ot, m = pending_proj[0]
                        if qb < (c + 1) * CHQ + 5:
                            return
                        pending_proj.pop(0)
                        _emit_proj_m(c, ot, m, pps)

                state = {}

                kgs = {}

                def emit_kg(qb):
                    # kg is prefetched one iteration ahead of its scores and
                    # emitted FIRST so it leads the in-order gpsimd queue
                    kg = gp.tile([128, TOPK, BLK], F16, tag="kg",
                                 name=f"kg_{qb}")
                    nc.gpsimd.ap_gather(kg[:], kT[:], kidx[:, qb:qb + 1],
                                        channels=128, num_elems=NB, d=BLK, num_idxs=TOPK)
                    kgs[qb] = kg

                def emit_vg(qb):
                    vg0 = gp.tile([128, TOPK, 66], F16, tag="vg0",
                                  name=f"vg0_{qb}")
                    nc.gpsimd.ap_gather(vg0[:], v0[:], vidx0[:, qb:qb + 1],
                                        channels=128, num_elems=NB, d=66, num_idxs=TOPK)
                    vg1 = gp.tile([128, TOPK, 66], F16, tag="vg1",
                                  name=f"vg1_{qb}")
                    nc.gpsimd.ap_gather(vg1[:], v1[:], vidx1[:, qb:qb + 1],
                                        channels=128, num_elems=NB, d=66, num_idxs=TOPK)
                    onorm = ob.tile([128, 2 * D], F16, tag="onorm",
                                    name=f"on_{qb}")
                    state[qb] = ([[None] * 4, [None] * 4], kgs.pop(qb), vg0, vg1,
                                 onorm)

                def emit_scores_quarter(qb, quarter):
                    # one K=128 MM per selected block covers BOTH heads via the
                    # block-diagonal qzzT rhs: out cols 0:128=h0, 128:256=h1.
                    # (K=64 lhsT weights don't get FWL; this keeps the PE at
                    # the N=256 streaming rate instead of the 107ns LDW rate.)
                    etiles, kg, vg0, vg1, onorm = state[qb]
                    s = spp.tile([128, 4, 256], F32, tag="s",
                                 name=f"s_{qb}_{quarter}")
                    for jj in range(4):
                        j = quarter * 4 + jj
                        nc.tensor.matmul(s[:, jj, :], lhsT=kg[:, j, :],
                                         rhs=qzzT[:, qb, :],
                                         start=True, stop=True)
                    e0 = ep.tile([128, 4, 128], F16, tag="e",
                                 name=f"e0_{qb}_{quarter}")
                    nc.scalar.activation(e0[:], s[:, :, 0:128],
                                         mybir.ActivationFunctionType.Exp, scale=SCALE)
                    e1 = ep.tile([128, 4, 128], F16, tag="e",
                                 name=f"e1_{qb}_{quarter}")
                    if quarter in (0, 2):
                        # the scores PSUM ring (2 bufs) makes quarter i+2 wait
                        # on BOTH exps of quarter i; running e1 of the even
                        # quarters on DVE (Schraudolph) pair-parallel with
                        # ACT's e0 halves that gate latency; at these emission
                        # points the DVE queue is idle so nothing head-blocks
                        # (offloading the odd quarters too measured SLOWER:
                        # their schrau head-blocks reciprocal/normalize)
                        nc.vector.tensor_scalar(
                            e1[:].bitcast(I16), s[:, :, 128:256],
                            SCHRAU_A, SCHRAU_B,
                            op0=mybir.AluOpType.mult, op1=mybir.AluOpType.add)
                    else:
                        nc.scalar.activation(e1[:], s[:, :, 128:256],
                                             mybir.ActivationFunctionType.Exp,
                                             scale=SCALE)
                    etiles[1][quarter] = e1
                    etiles[0][quarter] = e0

                def emit_o(qb, heads=(0, 1)):
                    if qb not in state:
                        return
                    etiles, kg, vg0, vg1, onorm = state[qb]
                    for h in heads:
                        vg = vg0 if h == 0 else vg1
                        o_ps = opp.tile([128, D + 1], F32, tag="o",
                                        name=f"o_{qb}_{h}")
                        for j in range(TOPK):
                            nc.tensor.matmul(o_ps[:],
                                             lhsT=etiles[h][j // 4][:, j % 4, :],
                                             rhs=vg[:, j, 0:D + 1],
                                             start=(j == 0), stop=(j == TOPK - 1))
                        rec = ob.tile([128, 1], F32, tag="rec", name=f"r_{qb}_{h}")
                        nc.vector.reciprocal(rec[:], o_ps[:, D:D + 1])
                        nc.vector.tensor_scalar(onorm[:, h * D:(h + 1) * D],
                                                o_ps[:, 0:D], rec[:], None,
                                                op0=mybir.AluOpType.mult)
                    if heads[-1] == 1:
                        state.pop(qb)
                        onorms[qb] = onorm

                # software pipeline: o-phase trails scores by one iteration and
                # is interleaved BETWEEN the two score halves so attn-v matmuls
                # cover the exp latency (sps bufs=2 reuses the half-0 PSUM
                # banks for half 1 only after its exp drains them)
                # software pipeline: kg leads by 1 iteration, the o-phase
                # trails by 3 and the o^T PE transpose by 5, giving the DVE
                # tail (schrau e1 / normalize) a full extra iteration of
                # slack before the PE consumes its outputs
                with tc.tile_pool(name="sps", bufs=2, space="PSUM") as spp, \
                     tc.tile_pool(name="ops", bufs=2, space="PSUM") as opp, \
                     tc.tile_pool(name="pps", bufs=1, space="PSUM") as pps, \
                     tc.tile_pool(name="trp", bufs=1, space="PSUM") as trp:
                    emit_kg(0)
                    for qb in range(NB):
                        if qb + 1 < NB:
                            emit_kg(qb + 1)
                        emit_vg(qb)
                        emit_scores_quarter(qb, 0)
                        emit_scores_quarter(qb, 1)
                        emit_o(qb - 3, heads=(0,))
                        emit_tr(qb - 5)
                        _pop_proj(1, qb)
                        emit_scores_quarter(qb, 2)
                        emit_scores_quarter(qb, 3)
                        emit_o(qb - 3, heads=(1,))
                        _pop_proj(1, qb)
                    _pop_proj(len(pending_proj), NB - 1)
                    for qb in (NB - 3, NB - 2, NB - 1):
                        emit_o(qb, heads=(0,))
                        emit_o(qb, heads=(1,))
                    for qb in range(NB - 5, NB):
                        emit_tr(qb)

                # the in-loop PSUM pools are closed now; drain the remaining
                # proj m-tiles through a 4-deep ring with the PSUM->SBUF
                # copies alternating DVE/ACT so the tail is copy-throughput
                # bound on two engines instead of a serial 1-buffer chain
                with tc.tile_pool(name="dps", bufs=4, space="PSUM") as dps:
                    for di in range(len(pending_proj)):
                        c, ot, m = pending_proj.pop(0)
                        _emit_proj_m(c, ot, m, dps,
                                     copy_eng=(nc.scalar.copy if di % 2
                                               else nc.vector.tensor_copy))

    nc.compile()
    return nc


def _prep_inputs(x, qkv_w, proj_w, proj_b):
    x = np.asarray(x, dtype=np.float32)
    qkv_w = np.asarray(qkv_w, dtype=np.float32)
    proj_w = np.asarray(proj_w, dtype=np.float32)
    proj_b = np.asarray(proj_b, dtype=np.float32)

    xT = x[0].T.astype(np.float16)                         # [C, N]
    # xT.reshape(KC_a, 128_p, 8_nch, 512_m) -> [nch, p, a, m] partition-major
    xT5 = np.ascontiguousarray(
        xT.reshape(8, 128, 8, 512).transpose(2, 1, 0, 3))
    ident64 = np.eye(64, dtype=np.float32)
    ident128 = np.eye(128, dtype=np.float16)
    erep = (np.arange(128)[None, :] % 16 == np.arange(16)[:, None]).astype(np.float32)
    in_maps = []
    for i in range(NCORES):
        h0 = HPC * i
        rows = []
        for part in range(3):                              # q, k, v row groups
            base = part * C + h0 * D
            rows.append(qkv_w[base:base + HPC * D, :])
        wqkv = np.concatenate(rows, axis=0)                # [384, C]
        wqkvT_np = np.ascontiguousarray(wqkv.T)            # [C, 384]
        wq5 = np.ascontiguousarray(
            wqkvT_np.reshape(8, 128, 384).transpose(1, 0, 2))
        cslice = slice(i * 2 * D, (i + 1) * 2 * D)
        in_maps.append({
            "xT5": xT5,
            "wqkvT": wq5.astype(np.float16),
            # [c_local, j]: rows = this core's 128 c-dims, cols = all 1024 j
            "projWT": np.ascontiguousarray(proj_w[:, cslice].T).astype(np.float16),
            "ident64": ident64,
            "ident128": ident128,
            "erep": erep,
        })
    return in_maps, proj_b


def kernel(x, qkv_w, proj_w, proj_b, _trace=False):
    if "nc" not in _CACHE:
        _CACHE["nc"] = _build()
    nc = _CACHE["nc"]
    in_maps, bias = _prep_inputs(x, qkv_w, proj_w, proj_b)
    res = run_bass_kernel_spmd(nc, in_maps, core_ids=list(range(NCORES)),
                               trace=_trace)
    outT = res.results[0]["out"].astype(np.float32)
    for i in range(1, NCORES):
        outT += res.results[i]["out"].astype(np.float32)
    out = np.ascontiguousarray(outT.T).reshape(1, N, C).astype(np.float32)
    out += bias[None, None, :]
    if _trace:
        _CACHE["last_exec_time_ns"] = res.exec_time_ns
        _CACHE["last_results"] = res
    return out

